# revision 32
# baseline (speedup 1.0000x reference)
"""Trainium2 Bass kernel for nn_CombinedLoss (MSE + pairwise margin ranking + cosine).

Math
----
total = 0.9*mse + 0.1*margin + 0.1*(1 - mean(cos))

The O(N^2) part is the pairwise margin ranking loss over i<j:
    hinge_ij = relu(-r_ij * dy_ij),  r = sign(dl) (sign(dy) on label ties)
hinge is symmetric in (i,j) and zero on the diagonal, and
    hinge_ij = (|dy| - sign(dl)*dy)/2   for all cases,
so with S_relu = sum_{ij} relu(y_i - y_j)  (= sum|dy|/2 by i<->j symmetry)
and  g_i = sum_j sign(l_i - l_j):
    sum_{all ordered pairs} hinge = S_relu - sum_i y_i*g_i
    margin = (S_relu - sum_i y_i*g_i) / (N*(N-1))

Device work, sharded as 4 row-shards x 2 column-blocks over 8 cores:
  * per-row sums of max(y_i, y_f) over the core's column block (DVE
    tensor_scalar with fused accumulate; relu row-sum = that minus
    sum_f y_f), or direct relu row-sums on ACT,
  * per-row counts #{f: l_f < l_i} (DVE) or direct sign row-sums (ACT),
  * per-partition partial sums of (y-l)^2 and cos terms (DVE).
Column vectors are broadcast to 128 partitions in float16 (error ~1e-7
on the final scalar); row operands stay exact float32.

Host only shards inputs, sums the per-core accumulators in float64 and
applies the closed-form combination above.
"""

import numpy as np

N = 8192
NCORES = 8
RHALVES = 4          # row shards
CBLOCKS = 2          # column blocks
ROWS = N // RHALVES  # 4096 rows per core
COLS = N // CBLOCKS  # 2048 cols per core
T = ROWS // 128      # 32 row tiles per core
MSEC = (N // NCORES) // 128  # 8 free-dim cols of the per-core mse/cos slice

ALPHA, BETA, GAMMA, EPS = 0.9, 0.1, 0.1, 1e-8

# Engine assignment for the 2*T accumulation passes (per core).
# ACT processes ~1 column per 1.0ns vs DVE ~0.3ns (f16 4x mode), so ACT
# takes ~1/4 of the passes.
ACT_Y = (3, 7, 11, 15)       # relu stream tiles on ACT
ACT_L = (1, 5, 9)            # sign stream tiles on ACT (7 total balances DVE)
DVE_Y = tuple(t for t in range(T) if t not in ACT_Y)
DVE_L = tuple(t for t in range(T) if t not in ACT_L)

_NC_CACHE = {}


def build_nc():
    """Build the (SPMD, per-core) Bass program. Same NEFF on all 8 cores."""
    import concourse.bacc as bacc
    import concourse.tile as tile
    from concourse import mybir

    f32 = mybir.dt.float32
    f16 = mybir.dt.float16
    Alu = mybir.AluOpType
    Act = mybir.ActivationFunctionType

    # Bacc (not raw Bass): its compile() pass lowers multi-semaphore waits
    # into legal instruction sequences; raw Bass BIR fails walrus codegen
    # with "Too many sync wait commands".
    nc = bacc.Bacc("TRN2", target_bir_lowering=False, debug=False, num_devices=NCORES)

    # Single input and single output DMA. The kernel-tail drain has only
    # ~4 sync-wait slots (ACT sem + DVE sem + one per DMA lane), and a DMA
    # instruction itself has a single wait slot, so we pack everything into
    # one input tensor (f32 rows bitcast into the f16 buffer, plus the
    # column broadcast pre-replicated across partitions) and one output.
    NR = 2 * T + 2 * MSEC                 # f32 row-operand columns
    NI = 2 * NR + 2 * COLS                # f16 columns of the input tile
    na = len(ACT_Y) + len(ACT_L)
    nd = len(DVE_Y) + len(DVE_L) + 2
    inp = nc.dram_tensor("inp", [128, NI], f16, kind="ExternalInput").ap()
    o_all = nc.dram_tensor("o_all", [128, nd + na], f32, kind="ExternalOutput").ap()

    with tile.TileContext(nc) as tc, tc.tile_pool(name="p", bufs=1) as pool:

        # --- input loading, ordered for the critical path ---
        # the tiny row-operand chunk first (lets the mse/cos chain start at
        # once), then the y-column broadcast, then the l-columns, each split
        # across parallel HWDGE queues.
        inp_s = pool.tile([128, NI], f16)
        lbase = 2 * NR + COLS
        nc.sync.dma_start(inp_s[:, 0:2 * NR], inp[:, 0:2 * NR])
        ysplit = 2 * NR + COLS // 2
        nc.sync.dma_start(inp_s[:, 2 * NR:ysplit], inp[:, 2 * NR:ysplit])
        nc.sync.dma_start(inp_s[:, ysplit:lbase], inp[:, ysplit:lbase])
        rm_s = inp_s[:, 0:2 * NR].bitcast(f32)
        yr_s = rm_s[:, 0:T]
        lr_s = rm_s[:, T:2 * T]
        ym_s = rm_s[:, 2 * T:2 * T + MSEC]
        lm_s = rm_s[:, 2 * T + MSEC:2 * T + 2 * MSEC]
        ycb = inp_s[:, 2 * NR:lbase]
        lcb = inp_s[:, lbase:NI]

        nc.sync.dma_start(inp_s[:, lbase:lbase + COLS // 2], inp[:, lbase:lbase + COLS // 2])
        nc.sync.dma_start(inp_s[:, lbase + COLS // 2:NI], inp[:, lbase + COLS // 2:NI])

        # --- accumulators: single output staging tile (disjoint per-engine
        # column ranges; Tile's AP-level dep tracking keeps the engines
        # independent, and Bacc lowers the output DMA's two waits) ---
        stage_d = pool.tile([128, nd + na], f32)
        acc_ya = stage_d[:, nd:nd + len(ACT_Y)]
        acc_la = stage_d[:, nd + len(ACT_Y):nd + na]
        acc_yd = stage_d[:, 0:len(DVE_Y)]
        acc_ld = stage_d[:, len(DVE_Y):len(DVE_Y) + len(DVE_L)]
        sm = stage_d[:, len(DVE_Y) + len(DVE_L):nd]
        scr_a = pool.tile([128, COLS], f16)   # ACT elementwise dump
        scr_d = pool.tile([128, COLS], f16)   # DVE elementwise dump

        # --- mse + cosine partials on the core's 1/8 slice (all DVE) ---
        d = pool.tile([128, MSEC], f32)
        nc.vector.tensor_sub(d[:], ym_s[:], lm_s[:])
        sq = pool.tile([128, MSEC], f32)
        nc.vector.tensor_mul(sq[:], d[:], d[:])
        ssq = pool.tile([128, MSEC], f32)
        nc.vector.tensor_scalar(
            out=ssq[:], in0=sq[:], scalar1=0.0, scalar2=None,
            op0=Alu.add, op1=Alu.add, accum_out=sm[:, 0:1],
        )
        p = pool.tile([128, MSEC], f32)
        nc.vector.tensor_mul(p[:], ym_s[:], lm_s[:])
        # max(|x|, EPS): negate, elementwise max, clamp
        nym = pool.tile([128, MSEC], f32)
        nc.vector.tensor_scalar_mul(nym[:], ym_s[:], -1.0)
        aym = pool.tile([128, MSEC], f32)
        nc.vector.tensor_max(aym[:], ym_s[:], nym[:])
        m1 = pool.tile([128, MSEC], f32)
        nc.vector.tensor_scalar_max(m1[:], aym[:], EPS)
        nlm = pool.tile([128, MSEC], f32)
        nc.vector.tensor_scalar_mul(nlm[:], lm_s[:], -1.0)
        alm = pool.tile([128, MSEC], f32)
        nc.vector.tensor_max(alm[:], lm_s[:], nlm[:])
        m2 = pool.tile([128, MSEC], f32)
        nc.vector.tensor_scalar_max(m2[:], alm[:], EPS)
        den = pool.tile([128, MSEC], f32)
        nc.vector.tensor_mul(den[:], m1[:], m2[:])
        rden = pool.tile([128, MSEC], f32)
        nc.vector.reciprocal(rden[:], den[:])
        cosv = pool.tile([128, MSEC], f32)
        nc.vector.tensor_mul(cosv[:], p[:], rden[:])
        csc = pool.tile([128, MSEC], f32)
        nc.vector.tensor_scalar(
            out=csc[:], in0=cosv[:], scalar1=0.0, scalar2=None,
            op0=Alu.add, op1=Alu.add, accum_out=sm[:, 1:2],
        )

        # --- the 2*T big accumulation passes ---
        for k, t in enumerate(ACT_Y):
            # relu(y_i - yhat_f) row-sums, exact relu on ACT
            nc.scalar.activation(
                out=scr_a[:], in_=ycb[:], func=Act.Relu,
                bias=yr_s[:, t:t + 1], scale=-1.0,
                accum_out=acc_ya[:, k:k + 1],
            )
        for k, t in enumerate(ACT_L):
            # sum_f sign(l_i - lhat_f)
            nc.scalar.activation(
                out=scr_a[:], in_=lcb[:], func=Act.Sign,
                bias=lr_s[:, t:t + 1], scale=-1.0,
                accum_out=acc_la[:, k:k + 1],
            )
        for k, t in enumerate(DVE_Y):
            # sum_f max(y_i, yhat_f); relu row-sum = this - sum_f yhat_f
            nc.vector.tensor_scalar(
                out=scr_d[:], in0=ycb[:], scalar1=yr_s[:, t:t + 1],
                scalar2=None, op0=Alu.max, op1=Alu.add,
                accum_out=acc_yd[:, k:k + 1],
            )
        # the y-stream accumulators are complete here; ship them while the
        # l-stream is still running
        nc.sync.dma_start(o_all[:, 0:len(DVE_Y)], acc_yd[:])
        for k, t in enumerate(DVE_L):
            # #{f : lhat_f < l_i}
            nc.vector.tensor_scalar(
                out=scr_d[:], in0=lcb[:], scalar1=lr_s[:, t:t + 1],
                scalar2=None, op0=Alu.is_lt, op1=Alu.add,
                accum_out=acc_ld[:, k:k + 1],
            )

        nc.sync.dma_start(o_all[:, len(DVE_Y):nd + na], stage_d[:, len(DVE_Y):nd + na])

    nc.compile()
    return nc


def make_in_maps(y, l):
    """Shard full [N] y/labels into the 8 per-core input maps."""
    y = np.ascontiguousarray(y, dtype=np.float32).reshape(N)
    l = np.ascontiguousarray(l, dtype=np.float32).reshape(N)
    y16 = y.astype(np.float16)
    l16 = l.astype(np.float16)
    in_maps = []
    for c in range(NCORES):
        rh, cb = c // CBLOCKS, c % CBLOCKS
        rsl = slice(ROWS * rh, ROWS * rh + ROWS)
        csl = slice(COLS * cb, COLS * cb + COLS)
        msl = slice(1024 * c, 1024 * c + 1024)
        rm = np.ascontiguousarray(np.concatenate(
            [
                y[rsl].reshape(T, 128).T,
                l[rsl].reshape(T, 128).T,
                y[msl].reshape(MSEC, 128).T,
                l[msl].reshape(MSEC, 128).T,
            ],
            axis=1,
        ))
        nr = rm.shape[1]
        cc = np.concatenate([y16[csl], l16[csl]])
        inp = np.empty((128, 2 * nr + cc.size), np.float16)
        inp[:, 0:2 * nr] = rm.view(np.float16)
        inp[:, 2 * nr:] = cc[None, :]
        in_maps.append({"inp": inp})
    return in_maps


def combine(y, results):
    """float64 host combination of the per-core accumulators."""
    y = np.asarray(y, dtype=np.float32).reshape(N).astype(np.float64)

    y16 = np.asarray(y, dtype=np.float32).reshape(N).astype(np.float16)
    s_yhat_blocks = [
        y16[COLS * b:COLS * b + COLS].astype(np.float64).sum() for b in range(CBLOCKS)
    ]
    S_relu = 0.0
    G = np.zeros(N, dtype=np.float64)
    sum_sq = 0.0
    sum_cos = 0.0
    for c in range(NCORES):
        r = results[c]
        rh = c // CBLOCKS
        base = ROWS * rh
        na_y, na_l = len(ACT_Y), len(ACT_L)
        nd_y, nd_l = len(DVE_Y), len(DVE_L)
        o = r["o_all"]
        o_yd = o[:, 0:nd_y].astype(np.float64)
        o_ld = o[:, nd_y:nd_y + nd_l].astype(np.float64)
        sm = o[:, nd_y + nd_l:nd_y + nd_l + 2]
        o_ya = o[:, nd_y + nd_l + 2:nd_y + nd_l + 2 + na_y].astype(np.float64)
        o_la = o[:, nd_y + nd_l + 2 + na_y:].astype(np.float64)
        cb = c % CBLOCKS
        s_yhat = s_yhat_blocks[cb]
        S_relu += o_ya.sum()
        S_relu += o_yd.sum() - o_yd.shape[1] * 128.0 * s_yhat
        for k, t in enumerate(ACT_L):
            rows = slice(base + 128 * t, base + 128 * t + 128)
            G[rows] += o_la[:, k]
        for k, t in enumerate(DVE_L):
            rows = slice(base + 128 * t, base + 128 * t + 128)
            G[rows] += 2.0 * o_ld[:, k] - COLS
        sum_sq += sm[:, 0].astype(np.float64).sum()
        sum_cos += sm[:, 1].astype(np.float64).sum()

    G += 1.0  # diagonal sign(0)=0 correction for the count formulation

    S_sig = (y * G).sum()
    margin = (S_relu - S_sig) / (N * (N - 1.0))
    mse = sum_sq / N
    sim = 1.0 - sum_cos / N
    return np.float32(ALPHA * mse + BETA * margin + GAMMA * sim)


def kernel(y, labels):
    from concourse.bass_utils import run_bass_kernel_spmd

    y = np.asarray(y, dtype=np.float32)
    labels = np.asarray(labels, dtype=np.float32)

    if "nc" not in _NC_CACHE:
        _NC_CACHE["nc"] = build_nc()
    nc = _NC_CACHE["nc"]

    in_maps = make_in_maps(y, labels)
    try:
        res = run_bass_kernel_spmd(nc, in_maps, core_ids=list(range(NCORES)))
    except Exception:
        # one retry for transient tunnel/runtime failures
        res = run_bass_kernel_spmd(nc, in_maps, core_ids=list(range(NCORES)))
    out = combine(y, res.results)
    return np.asarray(out, dtype=np.float32)


# revision 34
# speedup vs baseline: 1.0299x; 1.0299x over previous
"""Trainium2 Bass kernel for nn_CombinedLoss (MSE + pairwise margin ranking + cosine).

Math
----
total = 0.9*mse + 0.1*margin + 0.1*(1 - mean(cos))

The O(N^2) part is the pairwise margin ranking loss over i<j:
    hinge_ij = relu(-r_ij * dy_ij),  r = sign(dl) (sign(dy) on label ties)
hinge is symmetric in (i,j) and zero on the diagonal, and
    hinge_ij = (|dy| - sign(dl)*dy)/2   for all cases,
so with S_relu = sum_{ij} relu(y_i - y_j)  (= sum|dy|/2 by i<->j symmetry)
and  g_i = sum_j sign(l_i - l_j):
    sum_{all ordered pairs} hinge = S_relu - sum_i y_i*g_i
    margin = (S_relu - sum_i y_i*g_i) / (N*(N-1))

Device work, sharded as 4 row-shards x 2 column-blocks over 8 cores:
  * per-row sums of max(y_i, y_f) over the core's column block (DVE
    tensor_scalar with fused accumulate; relu row-sum = that minus
    sum_f y_f), or direct relu row-sums on ACT,
  * per-row counts #{f: l_f < l_i} (DVE) or direct sign row-sums (ACT),
  * per-partition partial sums of (y-l)^2 and cos terms (DVE).
Column vectors are broadcast to 128 partitions in float16 (error ~1e-7
on the final scalar); row operands stay exact float32.

Host only shards inputs, sums the per-core accumulators in float64 and
applies the closed-form combination above.
"""

import numpy as np

N = 8192
NCORES = 8
RHALVES = 4          # row shards
CBLOCKS = 2          # column blocks
ROWS = N // RHALVES  # 4096 rows per core
COLS = N // CBLOCKS  # 2048 cols per core
T = ROWS // 128      # 32 row tiles per core
MSEC = (N // NCORES) // 128  # 8 free-dim cols of the per-core mse/cos slice

ALPHA, BETA, GAMMA, EPS = 0.9, 0.1, 0.1, 1e-8

# Engine assignment for the accumulation passes, as slots (tile, c0, c1).
# ACT processes ~1 column per 1.0ns vs DVE ~0.3ns (f16 4x mode), so ACT
# takes ~1/4 of the column work. The first pass of each engine is split at
# the DMA chunk boundary (COLS//2) so compute starts when half of the
# y-broadcast has landed; tile 14's columns are split fractionally between
# the engines to even out the remaining imbalance.
SPLIT_C = 2048
ACT_Y_SLOTS = ((3, 0, COLS // 2), (3, COLS // 2, COLS),
               (7, 0, COLS), (11, 0, COLS), (15, 0, COLS), (14, 0, SPLIT_C))
ACT_L_SLOTS = ((1, 0, COLS), (5, 0, COLS), (9, 0, COLS))
DVE_Y_SLOTS = ((0, 0, COLS // 2), (0, COLS // 2, COLS)) + tuple(
    (t, 0, COLS) for t in range(T) if t not in (0, 3, 7, 11, 15, 14)
) + ((14, SPLIT_C, COLS),)
DVE_L_SLOTS = tuple((t, 0, COLS) for t in range(T) if t not in (1, 5, 9))

_NC_CACHE = {}


def build_nc():
    """Build the (SPMD, per-core) Bass program. Same NEFF on all 8 cores."""
    import concourse.bacc as bacc
    import concourse.tile as tile
    from concourse import mybir

    f32 = mybir.dt.float32
    f16 = mybir.dt.float16
    Alu = mybir.AluOpType
    Act = mybir.ActivationFunctionType

    # Bacc (not raw Bass): its compile() pass lowers multi-semaphore waits
    # into legal instruction sequences; raw Bass BIR fails walrus codegen
    # with "Too many sync wait commands".
    nc = bacc.Bacc("TRN2", target_bir_lowering=False, debug=False, num_devices=NCORES)

    # Single input and single output DMA. The kernel-tail drain has only
    # ~4 sync-wait slots (ACT sem + DVE sem + one per DMA lane), and a DMA
    # instruction itself has a single wait slot, so we pack everything into
    # one input tensor (f32 rows bitcast into the f16 buffer, plus the
    # column broadcast pre-replicated across partitions) and one output.
    NR = 2 * T + 2 * MSEC                 # f32 row-operand columns
    NI = 2 * NR + 2 * COLS                # f16 columns of the input tile
    na = len(ACT_Y_SLOTS) + len(ACT_L_SLOTS)
    nd = len(DVE_Y_SLOTS) + len(DVE_L_SLOTS) + 2
    inp = nc.dram_tensor("inp", [128, NI], f16, kind="ExternalInput").ap()
    o_all = nc.dram_tensor("o_all", [128, nd + na], f32, kind="ExternalOutput").ap()

    with tile.TileContext(nc) as tc, tc.tile_pool(name="p", bufs=1) as pool:

        # --- input loading, ordered for the critical path ---
        # the tiny row-operand chunk first (lets the mse/cos chain start at
        # once), then the y-column broadcast, then the l-columns, each split
        # across parallel HWDGE queues.
        inp_s = pool.tile([128, NI], f16)
        lbase = 2 * NR + COLS
        nc.sync.dma_start(inp_s[:, 0:2 * NR], inp[:, 0:2 * NR])
        ysplit = 2 * NR + COLS // 2
        nc.sync.dma_start(inp_s[:, 2 * NR:ysplit], inp[:, 2 * NR:ysplit])
        nc.sync.dma_start(inp_s[:, ysplit:lbase], inp[:, ysplit:lbase])
        rm_s = inp_s[:, 0:2 * NR].bitcast(f32)
        yr_s = rm_s[:, 0:T]
        lr_s = rm_s[:, T:2 * T]
        ym_s = rm_s[:, 2 * T:2 * T + MSEC]
        lm_s = rm_s[:, 2 * T + MSEC:2 * T + 2 * MSEC]
        ycb = inp_s[:, 2 * NR:lbase]
        lcb = inp_s[:, lbase:NI]

        nc.sync.dma_start(inp_s[:, lbase:lbase + COLS // 2], inp[:, lbase:lbase + COLS // 2])
        nc.sync.dma_start(inp_s[:, lbase + COLS // 2:NI], inp[:, lbase + COLS // 2:NI])

        # --- accumulators: single output staging tile (disjoint per-engine
        # column ranges; Tile's AP-level dep tracking keeps the engines
        # independent, and Bacc lowers the output DMA's two waits) ---
        stage_d = pool.tile([128, nd + na], f32)
        acc_ya = stage_d[:, nd:nd + len(ACT_Y_SLOTS)]
        acc_la = stage_d[:, nd + len(ACT_Y_SLOTS):nd + na]
        acc_yd = stage_d[:, 0:len(DVE_Y_SLOTS)]
        acc_ld = stage_d[:, len(DVE_Y_SLOTS):len(DVE_Y_SLOTS) + len(DVE_L_SLOTS)]
        sm = stage_d[:, len(DVE_Y_SLOTS) + len(DVE_L_SLOTS):nd]
        scr_a = pool.tile([128, COLS], f16)   # ACT elementwise dump
        scr_d = pool.tile([128, COLS], f16)   # DVE elementwise dump

        # --- mse + cosine partials on the core's 1/8 slice (all DVE) ---
        d = pool.tile([128, MSEC], f32)
        nc.vector.tensor_sub(d[:], ym_s[:], lm_s[:])
        sq = pool.tile([128, MSEC], f32)
        nc.vector.tensor_mul(sq[:], d[:], d[:])
        ssq = pool.tile([128, MSEC], f32)
        nc.vector.tensor_scalar(
            out=ssq[:], in0=sq[:], scalar1=0.0, scalar2=None,
            op0=Alu.add, op1=Alu.add, accum_out=sm[:, 0:1],
        )
        p = pool.tile([128, MSEC], f32)
        nc.vector.tensor_mul(p[:], ym_s[:], lm_s[:])
        # max(|x|, EPS): negate, elementwise max, clamp
        nym = pool.tile([128, MSEC], f32)
        nc.vector.tensor_scalar_mul(nym[:], ym_s[:], -1.0)
        aym = pool.tile([128, MSEC], f32)
        nc.vector.tensor_max(aym[:], ym_s[:], nym[:])
        m1 = pool.tile([128, MSEC], f32)
        nc.vector.tensor_scalar_max(m1[:], aym[:], EPS)
        nlm = pool.tile([128, MSEC], f32)
        nc.vector.tensor_scalar_mul(nlm[:], lm_s[:], -1.0)
        alm = pool.tile([128, MSEC], f32)
        nc.vector.tensor_max(alm[:], lm_s[:], nlm[:])
        m2 = pool.tile([128, MSEC], f32)
        nc.vector.tensor_scalar_max(m2[:], alm[:], EPS)
        den = pool.tile([128, MSEC], f32)
        nc.vector.tensor_mul(den[:], m1[:], m2[:])
        rden = pool.tile([128, MSEC], f32)
        nc.vector.reciprocal(rden[:], den[:])
        cosv = pool.tile([128, MSEC], f32)
        nc.vector.tensor_mul(cosv[:], p[:], rden[:])
        csc = pool.tile([128, MSEC], f32)
        nc.vector.tensor_scalar(
            out=csc[:], in0=cosv[:], scalar1=0.0, scalar2=None,
            op0=Alu.add, op1=Alu.add, accum_out=sm[:, 1:2],
        )

        # --- the big accumulation passes (slot-driven) ---
        for k, (t, c0, c1) in enumerate(ACT_Y_SLOTS):
            # relu(y_i - yhat_f) row-sums over [c0, c1), exact relu on ACT
            nc.scalar.activation(
                out=scr_a[:, c0:c1], in_=ycb[:, c0:c1], func=Act.Relu,
                bias=yr_s[:, t:t + 1], scale=-1.0,
                accum_out=acc_ya[:, k:k + 1],
            )
        for k, (t, c0, c1) in enumerate(ACT_L_SLOTS):
            # sum_f sign(l_i - lhat_f)
            nc.scalar.activation(
                out=scr_a[:, c0:c1], in_=lcb[:, c0:c1], func=Act.Sign,
                bias=lr_s[:, t:t + 1], scale=-1.0,
                accum_out=acc_la[:, k:k + 1],
            )
        for k, (t, c0, c1) in enumerate(DVE_Y_SLOTS):
            # sum_f max(y_i, yhat_f); relu row-sum = this - sum_f yhat_f
            nc.vector.tensor_scalar(
                out=scr_d[:, c0:c1], in0=ycb[:, c0:c1], scalar1=yr_s[:, t:t + 1],
                scalar2=None, op0=Alu.max, op1=Alu.add,
                accum_out=acc_yd[:, k:k + 1],
            )
        # the y-stream accumulators are complete here; ship them while the
        # l-stream is still running
        nc.sync.dma_start(o_all[:, 0:len(DVE_Y_SLOTS)], acc_yd[:])
        for k, (t, c0, c1) in enumerate(DVE_L_SLOTS):
            # #{f : lhat_f < l_i}
            nc.vector.tensor_scalar(
                out=scr_d[:, c0:c1], in0=lcb[:, c0:c1], scalar1=lr_s[:, t:t + 1],
                scalar2=None, op0=Alu.is_lt, op1=Alu.add,
                accum_out=acc_ld[:, k:k + 1],
            )

        nc.sync.dma_start(
            o_all[:, len(DVE_Y_SLOTS):nd + na],
            stage_d[:, len(DVE_Y_SLOTS):nd + na],
        )

    nc.compile()
    return nc


def make_in_maps(y, l):
    """Shard full [N] y/labels into the 8 per-core input maps."""
    y = np.ascontiguousarray(y, dtype=np.float32).reshape(N)
    l = np.ascontiguousarray(l, dtype=np.float32).reshape(N)
    y16 = y.astype(np.float16)
    l16 = l.astype(np.float16)
    in_maps = []
    for c in range(NCORES):
        rh, cb = c // CBLOCKS, c % CBLOCKS
        rsl = slice(ROWS * rh, ROWS * rh + ROWS)
        csl = slice(COLS * cb, COLS * cb + COLS)
        msl = slice(1024 * c, 1024 * c + 1024)
        rm = np.ascontiguousarray(np.concatenate(
            [
                y[rsl].reshape(T, 128).T,
                l[rsl].reshape(T, 128).T,
                y[msl].reshape(MSEC, 128).T,
                l[msl].reshape(MSEC, 128).T,
            ],
            axis=1,
        ))
        nr = rm.shape[1]
        cc = np.concatenate([y16[csl], l16[csl]])
        inp = np.empty((128, 2 * nr + cc.size), np.float16)
        inp[:, 0:2 * nr] = rm.view(np.float16)
        inp[:, 2 * nr:] = cc[None, :]
        in_maps.append({"inp": inp})
    return in_maps


def combine(y, results):
    """float64 host combination of the per-core accumulators."""
    y = np.asarray(y, dtype=np.float32).reshape(N).astype(np.float64)

    y16 = np.asarray(y, dtype=np.float32).reshape(N).astype(np.float16)

    def s_yhat(cb, c0, c1):
        return y16[COLS * cb + c0:COLS * cb + c1].astype(np.float64).sum()

    S_relu = 0.0
    G = np.zeros(N, dtype=np.float64)
    sum_sq = 0.0
    sum_cos = 0.0
    for c in range(NCORES):
        r = results[c]
        rh = c // CBLOCKS
        base = ROWS * rh
        na_y, na_l = len(ACT_Y_SLOTS), len(ACT_L_SLOTS)
        nd_y, nd_l = len(DVE_Y_SLOTS), len(DVE_L_SLOTS)
        o = r["o_all"]
        o_yd = o[:, 0:nd_y].astype(np.float64)
        o_ld = o[:, nd_y:nd_y + nd_l].astype(np.float64)
        sm = o[:, nd_y + nd_l:nd_y + nd_l + 2]
        o_ya = o[:, nd_y + nd_l + 2:nd_y + nd_l + 2 + na_y].astype(np.float64)
        o_la = o[:, nd_y + nd_l + 2 + na_y:].astype(np.float64)
        cb = c % CBLOCKS
        S_relu += o_ya.sum()
        for k, (t, c0, c1) in enumerate(DVE_Y_SLOTS):
            S_relu += o_yd[:, k].sum() - 128.0 * s_yhat(cb, c0, c1)
        for k, (t, c0, c1) in enumerate(ACT_L_SLOTS):
            rows = slice(base + 128 * t, base + 128 * t + 128)
            G[rows] += o_la[:, k]
        for k, (t, c0, c1) in enumerate(DVE_L_SLOTS):
            rows = slice(base + 128 * t, base + 128 * t + 128)
            G[rows] += 2.0 * o_ld[:, k] - (c1 - c0)
        sum_sq += sm[:, 0].astype(np.float64).sum()
        sum_cos += sm[:, 1].astype(np.float64).sum()

    G += 1.0  # diagonal sign(0)=0 correction for the count formulation

    S_sig = (y * G).sum()
    margin = (S_relu - S_sig) / (N * (N - 1.0))
    mse = sum_sq / N
    sim = 1.0 - sum_cos / N
    return np.float32(ALPHA * mse + BETA * margin + GAMMA * sim)


def kernel(y, labels):
    from concourse.bass_utils import run_bass_kernel_spmd

    y = np.asarray(y, dtype=np.float32)
    labels = np.asarray(labels, dtype=np.float32)

    if "nc" not in _NC_CACHE:
        _NC_CACHE["nc"] = build_nc()
    nc = _NC_CACHE["nc"]

    in_maps = make_in_maps(y, labels)
    try:
        res = run_bass_kernel_spmd(nc, in_maps, core_ids=list(range(NCORES)))
    except Exception:
        # one retry for transient tunnel/runtime failures
        res = run_bass_kernel_spmd(nc, in_maps, core_ids=list(range(NCORES)))
    out = combine(y, res.results)
    return np.asarray(out, dtype=np.float32)
